# revision 4
# baseline (speedup 1.0000x reference)
"""KMeans summarize kernel for Trainium2 — kernel7: SBUF bank separation.

Same algorithm as kernel2 (fp32r dist matmul, bf16 scatter matmul, Sign
one-hot), but every PE-read region is separated from concurrent writers:
ping-pong A/B pools for xt slabs, M slabs and a_t rings (writer parity !=
reader parity), spacer tiles between regions, mbuf isolated from rhs_sb.
Scatter matmuls flush in bursts of 4 with an 8-deep pend queue, so the PE
reads the a_t burst of opposite parity to the one ACT is writing.
"""

import sys

sys.path.insert(0, "/opt/trn_rl_repo")

import numpy as np

N, D, K = 500_000, 64, 512
NCORES = 8
P = 128
TILES = 489                      # per-core tiles
ROWS = P * TILES                 # 62592 per core
NPAD = NCORES * ROWS             # 500736
GROUP = 8
EPS = 2.0 ** -12                 # relative argmin margin
DIST_BUFS = 6
PEND_HI = 8                      # flush 4 scatter matmuls when 8 pending
FLUSH_N = 4
_GROUPS = [GROUP] * (TILES // GROUP) + ([TILES % GROUP] if TILES % GROUP else [])

_CACHE = {}


def _build(groups=None):
    import concourse.bass as bass
    import concourse.mybir as mybir
    import concourse.tile as tile

    if groups is None:
        groups = _GROUPS
    tiles = sum(groups)

    fp32 = mybir.dt.float32
    fp32r = mybir.dt.float32r
    bf16 = mybir.dt.bfloat16
    nc = bass.Bass()
    xt = nc.dram_tensor("xt", (D + 2, ROWS), fp32r, kind="ExternalInput")
    rhs = nc.dram_tensor("rhs", (D + 2, K), fp32r, kind="ExternalInput")
    mh = nc.dram_tensor("mh", (ROWS, 2 * D), bf16, kind="ExternalInput")
    out = nc.dram_tensor("out", (P, K + 2), fp32, kind="ExternalOutput")

    AX = mybir.AxisListType.X
    OP = mybir.AluOpType
    AF = mybir.ActivationFunctionType

    from contextlib import ExitStack

    with tile.TileContext(nc) as tc, ExitStack() as es:
        consts = es.enter_context(tc.tile_pool(name="consts", bufs=1))
        mpool = es.enter_context(tc.tile_pool(name="mpool", bufs=1))
        xtpA = es.enter_context(tc.tile_pool(name="xtpA", bufs=2))
        xtpB = es.enter_context(tc.tile_pool(name="xtpB", bufs=2))
        mgpA = es.enter_context(tc.tile_pool(name="mgpA", bufs=2))
        mgpB = es.enter_context(tc.tile_pool(name="mgpB", bufs=2))
        atpA = es.enter_context(tc.tile_pool(name="atpA", bufs=6))
        atpB = es.enter_context(tc.tile_pool(name="atpB", bufs=6))
        dpp = es.enter_context(tc.tile_pool(name="dpp", bufs=DIST_BUFS // 2, space="PSUM"))
        app = es.enter_context(tc.tile_pool(name="app", bufs=1, space="PSUM"))

        rhs_sb = consts.tile([D + 2, K], fp32r)
        nc.sync.dma_start(out=rhs_sb, in_=rhs.ap())
        sp0 = consts.tile([P, 4096], bf16, name="sp0")  # spacer after rhs_sb

        mbuf = mpool.tile([P, tiles], fp32)
        sp1 = mpool.tile([P, 4096], bf16, name="sp1")   # spacer after mbuf

        scat = app.tile([P, K], fp32)            # scatter accumulator (1 bank)

        pend = []          # (a_tile, mg, j, t) awaiting scatter matmul
        t = 0
        dist_d = None      # current PSUM pair tile [P, 2, K]
        sig_q = []         # (tile_t, dist_ap, a_t, mg, j) awaiting Sign issue
        for gi, g in enumerate(groups):
            # ---- group loads (ping-pong pools by group parity) ----
            xtg = (xtpA if gi % 2 == 0 else xtpB).tile(
                [D + 2, GROUP * P], fp32r, tag="xtg", name="xtg"
            )
            nc.sync.dma_start(
                out=xtg[:, : g * P], in_=xt.ap()[:, t * P:(t + g) * P]
            )
            mg = (mgpA if gi % 2 == 0 else mgpB).tile(
                [P, GROUP, 2 * D], bf16, tag="mg", name="mg"
            )
            src_m = bass.AP(
                tensor=mh, offset=t * P * 2 * D,
                ap=[[2 * D, P], [P * 2 * D, g], [1, 2 * D]],
            )
            nc.sync.dma_start(out=mg[:, :g, :], in_=src_m)

            for j in range(g):
                half = t % 2
                if half == 0:
                    dist_d = dpp.tile([P, 2, K], fp32, tag="dist")
                dist = dist_d[:, half, :]
                nc.tensor.matmul(
                    dist, xtg[:, j * P:(j + 1) * P], rhs_sb,
                    start=True, stop=True,
                )
                sig_q.append((t, dist, mg, j))
                if half == 1 or t == tiles - 1:
                    n = half + 1
                    nc.vector.tensor_reduce(
                        out=mbuf[:, t - n + 1:t + 1], in_=dist_d[:, :n, :],
                        axis=AX, op=OP.min,
                    )
                    for (st, sdist, smg, sj) in sig_q:
                        a_t = (atpA if (st // FLUSH_N) % 2 == 0 else atpB).tile(
                            [P, K], bf16, tag="a_t", name="a_t"
                        )
                        nc.scalar.activation(
                            out=a_t, in_=sdist, func=AF.Sign,
                            bias=mbuf[:, st:st + 1], scale=-(1.0 - EPS),
                        )
                        pend.append((a_t, smg, sj, st))
                        if len(pend) >= PEND_HI:
                            for _ in range(FLUSH_N):
                                pa, pm, pj, pt = pend.pop(0)
                                nc.tensor.matmul(
                                    scat, pm[:, pj, :], pa,
                                    start=(pt == 0), stop=(pt == tiles - 1),
                                )
                    sig_q = []
                t += 1
        for pa, pm, pj, pt in pend:
            nc.tensor.matmul(
                scat, pm[:, pj, :], pa,
                start=(pt == 0), stop=(pt == tiles - 1),
            )

        # ---- finalize ----
        out_sb = consts.tile([P, K + 2], fp32)
        t_sb = consts.tile([P, 1], fp32)
        nc.vector.tensor_reduce(out=t_sb, in_=scat, axis=AX, op=OP.add)
        nc.vector.tensor_scalar_mul(t_sb, t_sb, -1.0 / 510.0)
        # S = (scat + T) * 0.5
        nc.vector.tensor_scalar(
            out=out_sb[:, :K], in0=scat, scalar1=t_sb, scalar2=0.5,
            op0=OP.add, op1=OP.mult,
        )
        # inertia partials: sum sqrt(clip(m,0)/64)
        nc.vector.tensor_scalar_max(mbuf, mbuf, 0.0)
        sq_b = consts.tile([P, tiles], fp32)
        inert = consts.tile([P, 1], fp32)
        nc.scalar.activation(
            out=sq_b, in_=mbuf, func=AF.Sqrt, scale=1.0 / D, accum_out=inert
        )
        nc.vector.tensor_copy(out_sb[:, K:K + 1], inert)
        nc.sync.dma_start(out=out.ap(), in_=out_sb)
        del sp0, sp1

    _split_multi_waits(nc, mybir)
    return nc


def _split_multi_waits(nc, mybir):
    """This walrus build allows max 1 sem-wait per instruction: hoist extras
    onto inserted NoOps on the same engine queue."""
    import copy

    module = nc.m
    new_module = copy.replace(module, functions=[])
    for function in module.functions:
        new_function = copy.replace(function, blocks=[])
        new_function.set_allocations_from_list(function.allocations)
        for block in function.blocks:
            new_insts = []
            for ins in block.instructions:
                si = ins.sync_info
                if si is not None and si.on_wait and len(si.on_wait) > 1:
                    waits = list(si.on_wait)
                    for k, w in enumerate(waits[:-1]):
                        new_insts.append(mybir.InstNoOp(
                            name=f"{ins.name}-wsplit{k}", engine=ins.engine,
                            ins=[], outs=[],
                            sync_info=mybir.SyncInfo(on_wait=[w], on_update=[]),
                        ))
                    ins.sync_info = mybir.SyncInfo(
                        on_wait=[waits[-1]], on_update=list(si.on_update or [])
                    )
                new_insts.append(ins)
            new_function.blocks.append(copy.replace(block, instructions=new_insts))
        new_module.functions.append(new_function)
    nc.m = new_module


def _prep_inputs(X, centroids, sample_weight):
    import ml_dtypes

    C = np.asarray(centroids, dtype=np.float32)
    X = np.asarray(X, dtype=np.float32)
    W = np.asarray(sample_weight, dtype=np.float32)
    rhs = np.empty((D + 2, K), dtype=np.float32)
    rhs[:D] = -2.0 * C.T
    rhs[D] = (C * C).sum(axis=1)
    rhs[D + 1] = 1.0
    Xp = np.empty((NPAD, D), dtype=np.float32)
    Xp[:N] = X
    Xp[N:] = C[0]
    Wp = np.zeros((NPAD, D), dtype=np.float32)
    Wp[:N] = W
    XXp = np.einsum("ij,ij->i", Xp, Xp)
    Mh = np.empty((NPAD, 2 * D), dtype=ml_dtypes.bfloat16)
    Mh[:, :D] = Xp * Wp
    Mh[:, D:] = Wp
    in_maps = []
    for c in range(NCORES):
        sl = slice(c * ROWS, (c + 1) * ROWS)
        xtc = np.empty((D + 2, ROWS), dtype=np.float32)
        xtc[:D] = Xp[sl].T
        xtc[D] = 1.0
        xtc[D + 1] = XXp[sl]
        in_maps.append({"xt": xtc, "rhs": rhs, "mh": np.ascontiguousarray(Mh[sl])})
    return in_maps


def run(X, centroids, sample_weight, trace=False):
    from concourse.bass_utils import run_bass_kernel_spmd

    if "nc" not in _CACHE:
        _CACHE["nc"] = _build()
    in_maps = _prep_inputs(X, centroids, sample_weight)
    res = run_bass_kernel_spmd(
        _CACHE["nc"], in_maps, core_ids=list(range(NCORES)), trace=trace
    )
    xw = np.zeros((K, D), dtype=np.float64)
    ws = np.zeros((K, D), dtype=np.float64)
    inertia = 0.0
    for c in range(NCORES):
        o = res.results[c]["out"]
        xw += o[:D, :K].T.astype(np.float64)
        ws += o[D:2 * D, :K].T.astype(np.float64)
        inertia += float(o[:, K].sum(dtype=np.float64))
    packed = np.concatenate(
        [xw, ws, np.full((1, D), inertia)], axis=0
    ).astype(np.float32)
    return packed, res


def kernel(X, centroids, sample_weight):
    packed, _ = run(X, centroids, sample_weight)
    return packed
